# revision 22
# baseline (speedup 1.0000x reference)
"""DAG-LSTM Trainium2 kernel.

Problem: 2-layer LSTM scanned over a 48-node DAG, batch 1024, hidden 256.
Sharding: pure data parallelism -- batch split 8 x 128 across NeuronCores,
weights replicated, no cross-device traffic.

Key optimizations over the naive scan:
1. Dead-code elimination: the reference returns only the top-layer hidden
   state of the LAST DAG node, so only the ancestor cone of (node 47,
   layer 1) is computed -- 20 of 96 (node, layer) units for this graph.
2. The layer-0 input-side gate contributions (W_ih0 @ x_i) have no DAG
   dependency; they are precomputed in wide batched matmuls during the
   input-DMA head (also warming up the PE) and stashed in SBUF fp32.
   Per-group PSUM is then preloaded with bias + x-gates in one DVE op and
   the in-loop matmuls are h-side only (start=False accumulation).
3. Merged activations: one contiguous PSUM tile [128, 8 chunks, u*128] per
   group => sigmoid(i,f) / tanh(g) / sigmoid(o) are three instructions.
4. c-path kept fp32 (f16 c on HW costs ~1.3e-2 rel err; fp32 gives 9e-4).

Layout: "transposed" (feature-on-partition): states h [2x128 part-chunks,
B=128 free] f16, c fp32.
"""

import sys
import numpy as np

sys.path.insert(0, "/opt/trn_rl_repo")

B, N, IN, H, L, P = 1024, 48, 256, 256, 2, 2
NCORES = 8
BL = B // NCORES          # 128 batch per core
KC = 2                    # K chunks (256 = 2*128)
GROUP_MAX = 2             # units per (stage, layer) group
PCW = 2                   # precompute wave width (nodes per wave)

_CACHE = {}


def _unit_deps(pred, i, l):
    d = [(int(v) - 1, l) for v in pred[i] if v > 0]
    if l == 1:
        d.append((i, 0))
    return d


def _build_schedule(pred):
    """Ancestor cone of (N-1, 1) scheduled into ASAP stages; same-stage
    same-layer units grouped up to GROUP_MAX.  Returns list of
    (layer, [nodes]) in dependency order."""
    cone = set()
    stack = [(N - 1, 1)]
    while stack:
        u = stack.pop()
        if u in cone:
            continue
        cone.add(u)
        stack.extend(_unit_deps(pred, *u))
    stage = {}
    for u in sorted(cone):
        ds = [d for d in _unit_deps(pred, *u) if d in cone]
        stage[u] = max([stage[d] for d in ds], default=-1) + 1
    nstages = max(stage.values()) + 1
    groups = []
    for s in range(nstages):
        for l in (0, 1):
            nodes = sorted(i for (i, ll), st in stage.items()
                           if st == s and ll == l)
            for k in range(0, len(nodes), GROUP_MAX):
                groups.append((l, nodes[k:k + GROUP_MAX]))
    return groups


def _prep_weights(w_ih, w_hh):
    """Host-side weight prep -> [128, KC, 1024] fp16 stationary tiles.
    The 0.5 predecessor-mean is folded into W_hh."""

    def to_t(w):
        kdim = w.shape[1]
        wt = np.ascontiguousarray(w.T)            # [K, 1024]
        wt = wt.reshape(kdim // 128, 128, 1024)   # [kc, kin, 1024]
        return np.ascontiguousarray(
            wt.transpose(1, 0, 2).astype(np.float16))  # [128, kc, 1024]

    return to_t(w_ih), to_t(w_hh * 0.5)


def _build_program(pred):
    from contextlib import ExitStack
    from concourse import bacc, mybir, tile

    f32 = mybir.dt.float32
    f16 = mybir.dt.float16
    AF = mybir.ActivationFunctionType
    Alu = mybir.AluOpType

    groups = _build_schedule(pred)
    # layer-0 nodes in group order -> column order of the precomputed
    # x-gates stash (so per-group slices are contiguous)
    l0_nodes = [i for (l, nodes) in groups if l == 0 for i in nodes]
    l0_col = {i: k for k, i in enumerate(l0_nodes)}
    nl0 = len(l0_nodes)

    nc = bacc.Bacc("TRN2", target_bir_lowering=False, debug=False,
                   num_devices=NCORES)

    # dags gathered+transposed on host: [128, KC, nl0, 128] (group order)
    dags_t = nc.dram_tensor("dags_t", [128, KC, nl0, 128], f16,
                            kind="ExternalInput")
    h0_t = nc.dram_tensor("h0_t", [128, L, KC, 128], f16,
                          kind="ExternalInput")
    c0_t = nc.dram_tensor("c0_t", [128, L, KC, 128], f32,
                          kind="ExternalInput")
    w_dram = {}
    for l in range(L):
        w_dram[("x", l)] = nc.dram_tensor(f"wx{l}", [128, KC, 1024], f16,
                                          kind="ExternalInput")
        w_dram[("h", l)] = nc.dram_tensor(f"wh{l}", [128, KC, 1024], f16,
                                          kind="ExternalInput")
    # bias image pre-broadcast for PSUM preload: [128, L, 8, GROUP_MAX*128]
    bias_dram = nc.dram_tensor("bias_img", [128, L, 8, GROUP_MAX * 128], f16,
                               kind="ExternalInput")
    out_t = nc.dram_tensor("out_t", [KC, 128, 128], f32, kind="ExternalOutput")

    with tile.TileContext(nc) as tc, ExitStack() as ctx:
        consts = ctx.enter_context(tc.tile_pool(name="consts", bufs=1))
        ps = ctx.enter_context(tc.tile_pool(name="ps", bufs=2, space="PSUM"))
        gp = ctx.enter_context(tc.tile_pool(name="gp", bufs=2))

        # --- input DMAs, latency-critical first ---
        xall = consts.tile([128, KC, nl0, 128], f16, tag="xall")
        nc.sync.dma_start(out=xall[:], in_=dags_t[:])
        bias_sb = consts.tile([128, L, 8, GROUP_MAX * 128], f16, tag="bias")
        nc.sync.dma_start(out=bias_sb[:], in_=bias_dram[:])
        slot0_h = consts.tile([128, L, KC, 128], f16, tag="slot0h")
        nc.sync.dma_start(out=slot0_h[:], in_=h0_t[:])
        slot0_c = consts.tile([128, L, KC, 128], f32, tag="slot0c")
        nc.sync.dma_start(out=slot0_c[:], in_=c0_t[:])
        wsb = {}
        for key in [("x", 0), ("h", 0), ("h", 1), ("x", 1)]:
            t = consts.tile([128, KC, 1024], f16, tag=f"w{key[0]}{key[1]}",
                            name=f"w{key[0]}{key[1]}")
            nc.gpsimd.dma_start(out=t[:], in_=w_dram[key][:])
            wsb[key] = t

        # --- precompute bias + x-side gates for all layer-0 cone nodes ---
        # xg0[:, m, col*128 + b] = b0[m-chunk] + (W_ih0 @ x_col)[m-chunk]
        # (bias preloaded into psum; all matmuls accumulate, fp32 in SBUF)
        xg0 = consts.tile([128, 8, nl0 * 128], f32, tag="xg0")
        for w0 in range(0, nl0, PCW):
            w1 = min(w0 + PCW, nl0)
            wn = (w1 - w0) * 128
            pcp = ps.tile([128, 8, GROUP_MAX * 128], f32, tag="gates",
                          name="gates")
            nc.vector.tensor_copy(out=pcp[:, :, :wn],
                                  in_=bias_sb[:, 0, :, :wn])
            for m in range(8):
                for k in range(KC):
                    nc.tensor.matmul(
                        out=pcp[:, m, :wn],
                        lhsT=wsb[("x", 0)][:, k, m * 128:(m + 1) * 128],
                        rhs=xall[:, k, w0:w1].rearrange("p u b -> p (u b)"),
                        start=False, stop=(k == KC - 1),
                        skip_group_check=True)
            nc.scalar.copy(out=xg0[:, :, w0 * 128:w1 * 128], in_=pcp[:, :, :wn])

        # per-group persistent state tiles; units are views [:, j]
        st_h = {}
        st_c = {}
        for g, (l, nodes) in enumerate(groups):
            u = len(nodes)
            gh = consts.tile([128, u, KC, 128], f16, tag=f"gh{g}",
                             name=f"gh{g}")
            gc = consts.tile([128, u, KC, 128], f32, tag=f"gc{g}",
                             name=f"gc{g}")
            for j, i in enumerate(nodes):
                st_h[(i, l)] = gh[:, j]
                st_c[(i, l)] = gc[:, j]
            groups[g] = (l, nodes, gh, gc)
        outh = consts.tile([128, KC, 128], f32, tag="outh")

        def h_ap(v, l):
            if v == 0:
                return slot0_h[:, l]
            return st_h[(v - 1, l)]

        def c_ap(v, l):
            if v == 0:
                return slot0_c[:, l]
            return st_c[(v - 1, l)]

        for (l, nodes, gh, gc) in groups:
            u = len(nodes)
            un = u * 128
            # one contiguous psum tile for all 8 gate chunks
            pt = ps.tile([128, 8, GROUP_MAX * 128], f32, tag="gates",
                         name="gates")
            # PSUM preload: bias+x-gates (layer 0) or bias (layer 1)
            if l == 0:
                c0_ = l0_col[nodes[0]] * 128
                nc.vector.tensor_copy(out=pt[:, :, :un],
                                      in_=xg0[:, :, c0_:c0_ + un])
            else:
                nc.vector.tensor_copy(out=pt[:, :, :un],
                                      in_=bias_sb[:, l, :, :un])

            xq = gp.tile([128, KC, u, 128], f16, tag="xq", name="xq")
            ubh = gp.tile([128, KC, u, 128], f16, tag="ubh")
            ubc = gp.tile([128, KC, u, 128], f32, tag="ubc")
            sifo = gp.tile([128, 4, u, 128], f16, tag="sifo")
            gt = gp.tile([128, KC, u, 128], f16, tag="gt")
            so = gp.tile([128, KC, u, 128], f16, tag="so")
            vw = gp.tile([128, KC, u, 128], f16, tag="vw")
            cf = gp.tile([128, KC, u, 128], f32, tag="cf")
            th = gp.tile([128, u, KC, 128], f16, tag="th")

            # 1. layer-1 input: copy of h_l0
            if l == 1:
                for j, i in enumerate(nodes):
                    nc.vector.tensor_copy(out=xq[:, :, j, :],
                                          in_=st_h[(i, 0)])

            # 2. predecessor state sums
            for j, i in enumerate(nodes):
                a, b_ = int(pred[i][0]), int(pred[i][1])
                if a == b_:
                    nc.vector.tensor_scalar_mul(ubh[:, :, j, :], h_ap(a, l),
                                                2.0)
                    nc.vector.tensor_scalar_mul(ubc[:, :, j, :], c_ap(a, l),
                                                2.0)
                else:
                    nc.vector.tensor_tensor(out=ubh[:, :, j, :],
                                            in0=h_ap(a, l), in1=h_ap(b_, l),
                                            op=Alu.add)
                    nc.vector.tensor_tensor(out=ubc[:, :, j, :],
                                            in0=c_ap(a, l), in1=c_ap(b_, l),
                                            op=Alu.add)

            # 3. gate matmuls accumulate onto the preload (start=False).
            # Layer 0: h-side only (x-side precomputed).
            ops = [("h", ubh)] if l == 0 else [("h", ubh), ("x", xq)]
            for m in range(8):
                for oi, (kind, src) in enumerate(ops):
                    for k in range(KC):
                        last = (oi == len(ops) - 1 and k == KC - 1)
                        nc.tensor.matmul(
                            out=pt[:, m, :un],
                            lhsT=wsb[(kind, l)][:, k, m * 128:(m + 1) * 128],
                            rhs=src[:, k].rearrange("p u b -> p (u b)"),
                            start=False, stop=last, skip_group_check=True)

            # 4. merged activations: sigmoid(i,f), tanh(g), sigmoid(o)
            nc.scalar.activation(out=sifo[:].rearrange("p c u b -> p c (u b)"),
                                 in_=pt[:, 0:4, :un], func=AF.Sigmoid)
            nc.scalar.activation(out=gt[:].rearrange("p c u b -> p c (u b)"),
                                 in_=pt[:, 4:6, :un], func=AF.Tanh)
            nc.scalar.activation(out=so[:].rearrange("p c u b -> p c (u b)"),
                                 in_=pt[:, 6:8, :un], func=AF.Sigmoid)

            # 5. c_new = (ubc*0.5)*sigmoid(f) + sigmoid(i)*tanh(g)
            nc.vector.tensor_tensor(out=vw[:], in0=sifo[:, 0:2], in1=gt[:],
                                    op=Alu.mult)
            nc.vector.scalar_tensor_tensor(
                out=cf[:], in0=ubc[:], scalar=0.5, in1=sifo[:, 2:4],
                op0=Alu.mult, op1=Alu.mult)
            nc.vector.tensor_tensor(out=gc[:].rearrange("p u c b -> p c u b"),
                                    in0=cf[:], in1=vw[:], op=Alu.add)

            # 6. h = sigmoid(o) * tanh(c)
            nc.scalar.activation(out=th[:].rearrange("p u c b -> p (u c b)"),
                                 in_=gc[:].rearrange("p u c b -> p (u c b)"),
                                 func=AF.Tanh)
            nc.vector.tensor_tensor(out=gh[:].rearrange("p u c b -> p c u b"),
                                    in0=so[:],
                                    in1=th[:].rearrange("p u c b -> p c u b"),
                                    op=Alu.mult)
            if l == 1 and N - 1 in nodes:
                j = nodes.index(N - 1)
                nc.vector.tensor_tensor(out=outh[:], in0=so[:, :, j, :],
                                        in1=th[:, j], op=Alu.mult)

        # output: h of last node, top layer: [128, KC, 128] -> [KC, 128, 128]
        nc.sync.dma_start(out=out_t.ap().rearrange("k p b -> p k b"),
                          in_=outh[:])

    nc.compile()
    return nc, l0_nodes


def _get_program(pred):
    key = pred.tobytes()
    if key not in _CACHE:
        _CACHE[key] = _build_program(pred)
    return _CACHE[key]


def _prepare(dags, h0, c0, w_ih0, w_hh0, b_ih0, b_hh0,
             w_ih1, w_hh1, b_ih1, b_hh1, pred_idx):
    """Host-side prep: returns (nc, in_maps)."""
    dags = np.asarray(dags, dtype=np.float32)
    h0 = np.asarray(h0, dtype=np.float32)
    c0 = np.asarray(c0, dtype=np.float32)
    pred = np.asarray(pred_idx)

    nc, l0_nodes = _get_program(pred)

    wx0, wh0 = _prep_weights(np.asarray(w_ih0, np.float32),
                             np.asarray(w_hh0, np.float32))
    wx1, wh1 = _prep_weights(np.asarray(w_ih1, np.float32),
                             np.asarray(w_hh1, np.float32))
    bias = np.stack([
        np.asarray(b_ih0, np.float32) + np.asarray(b_hh0, np.float32),
        np.asarray(b_ih1, np.float32) + np.asarray(b_hh1, np.float32),
    ])  # [L, 1024]
    # bias image [128, L, 8, GROUP_MAX*128]: row (chunk*128 + p) broadcast
    bimg = np.ascontiguousarray(
        np.broadcast_to(
            bias.reshape(L, 8, 128).transpose(2, 0, 1)[:, :, :, None],
            (128, L, 8, GROUP_MAX * 128)).astype(np.float16))

    in_maps = []
    for c in range(NCORES):
        bs = slice(c * BL, (c + 1) * BL)
        # dags [B, N, IN] -> cone l0 nodes (group order) ->
        # [128(p), KC, n, 128(b)] fp16
        sel = dags[bs][:, l0_nodes]                      # [BL, n, IN]
        dt_ = sel.transpose(2, 1, 0).reshape(KC, 128, len(l0_nodes), BL)
        dt_ = np.ascontiguousarray(
            dt_.transpose(1, 0, 2, 3).astype(np.float16))
        # h0/c0 [L, B, H] -> [128(p), L, kc, b]
        hh = h0[:, bs, :].transpose(2, 0, 1).reshape(KC, 128, L, BL)
        cc = c0[:, bs, :].transpose(2, 0, 1).reshape(KC, 128, L, BL)
        h0t = np.ascontiguousarray(
            hh.transpose(1, 2, 0, 3).astype(np.float16))  # [128, L, kc, b]
        c0t = np.ascontiguousarray(
            cc.transpose(1, 2, 0, 3).astype(np.float32))
        in_maps.append({
            "dags_t": dt_, "h0_t": h0t, "c0_t": c0t,
            "wx0": wx0, "wh0": wh0, "wx1": wx1, "wh1": wh1,
            "bias_img": bimg,
        })
    return nc, in_maps


def _assemble(res):
    out = np.empty((B, H), np.float32)
    for c in range(NCORES):
        ot = res.results[c]["out_t"]  # [KC, 128, 128] = [kc, p, b]
        out[c * BL:(c + 1) * BL] = ot.reshape(H, BL).T
    return out


def kernel(**inputs):
    from concourse.bass_utils import run_bass_kernel_spmd

    nc, in_maps = _prepare(**inputs)
    res = run_bass_kernel_spmd(nc, in_maps, list(range(NCORES)))
    return _assemble(res)


# revision 24
# speedup vs baseline: 1.1271x; 1.1271x over previous
"""DAG-LSTM Trainium2 kernel.

Problem: 2-layer LSTM scanned over a 48-node DAG, batch 1024, hidden 256.
Sharding: pure data parallelism -- batch split 8 x 128 across NeuronCores,
weights replicated, no cross-device traffic.

Key optimizations over the naive scan:
1. Dead-code elimination: the reference returns only the top-layer hidden
   state of the LAST DAG node, so only the ancestor cone of (node 47,
   layer 1) is computed -- 20 of 96 (node, layer) units for this graph.
2. The layer-0 input-side gate contributions (W_ih0 @ x_i) have no DAG
   dependency; they are precomputed in wide batched matmuls during the
   input-DMA head (also warming up the PE) and stashed in SBUF fp32.
   Per-group PSUM is then preloaded with bias + x-gates in one DVE op and
   the in-loop matmuls are h-side only (start=False accumulation).
3. Merged activations: one contiguous PSUM tile [128, 8 chunks, u*128] per
   group => sigmoid(i,f) / tanh(g) / sigmoid(o) are three instructions.
4. c-path kept fp32 (f16 c on HW costs ~1.3e-2 rel err; fp32 gives 9e-4).

Layout: "transposed" (feature-on-partition): states h [2x128 part-chunks,
B=128 free] f16, c fp32.
"""

import sys
import numpy as np

sys.path.insert(0, "/opt/trn_rl_repo")

B, N, IN, H, L, P = 1024, 48, 256, 256, 2, 2
NCORES = 8
BL = B // NCORES          # 128 batch per core
KC = 2                    # K chunks (256 = 2*128)
GROUP_MAX = 2             # units per (stage, layer) group
PCW = 2                   # precompute wave width (nodes per wave)

_CACHE = {}


def _unit_deps(pred, i, l):
    d = [(int(v) - 1, l) for v in pred[i] if v > 0]
    if l == 1:
        d.append((i, 0))
    return d


def _build_schedule(pred):
    """Ancestor cone of (N-1, 1) scheduled into ASAP stages; same-stage
    same-layer units grouped up to GROUP_MAX.  Returns list of
    (layer, [nodes]) in dependency order."""
    cone = set()
    stack = [(N - 1, 1)]
    while stack:
        u = stack.pop()
        if u in cone:
            continue
        cone.add(u)
        stack.extend(_unit_deps(pred, *u))
    stage = {}
    for u in sorted(cone):
        ds = [d for d in _unit_deps(pred, *u) if d in cone]
        stage[u] = max([stage[d] for d in ds], default=-1) + 1
    nstages = max(stage.values()) + 1
    groups = []
    for s in range(nstages):
        for l in (0, 1):
            nodes = sorted(i for (i, ll), st in stage.items()
                           if st == s and ll == l)
            for k in range(0, len(nodes), GROUP_MAX):
                groups.append((l, nodes[k:k + GROUP_MAX]))
    return groups


def _prep_weights(w_ih, w_hh):
    """Host-side weight prep -> [128, KC, 1024] fp16 stationary tiles.
    The 0.5 predecessor-mean is folded into W_hh."""

    def to_t(w):
        kdim = w.shape[1]
        wt = np.ascontiguousarray(w.T)            # [K, 1024]
        wt = wt.reshape(kdim // 128, 128, 1024)   # [kc, kin, 1024]
        return np.ascontiguousarray(
            wt.transpose(1, 0, 2).astype(np.float16))  # [128, kc, 1024]

    return to_t(w_ih), to_t(w_hh * 0.5)


def _build_program(pred):
    from contextlib import ExitStack
    from concourse import bacc, mybir, tile

    f32 = mybir.dt.float32
    f16 = mybir.dt.float16
    AF = mybir.ActivationFunctionType
    Alu = mybir.AluOpType

    groups = _build_schedule(pred)
    # layer-0 nodes in group order -> column order of the precomputed
    # x-gates stash (so per-group slices are contiguous)
    l0_nodes = [i for (l, nodes) in groups if l == 0 for i in nodes]
    l0_col = {i: k for k, i in enumerate(l0_nodes)}
    nl0 = len(l0_nodes)

    nc = bacc.Bacc("TRN2", target_bir_lowering=False, debug=False,
                   num_devices=NCORES)

    # dags gathered+transposed on host: [128, KC, nl0, 128] (group order)
    dags_t = nc.dram_tensor("dags_t", [128, KC, nl0, 128], f16,
                            kind="ExternalInput")
    h0_t = nc.dram_tensor("h0_t", [128, L, KC, 128], f16,
                          kind="ExternalInput")
    c0_t = nc.dram_tensor("c0_t", [128, L, KC, 128], f32,
                          kind="ExternalInput")
    w_dram = {}
    for l in range(L):
        w_dram[("x", l)] = nc.dram_tensor(f"wx{l}", [128, KC, 1024], f16,
                                          kind="ExternalInput")
        w_dram[("h", l)] = nc.dram_tensor(f"wh{l}", [128, KC, 1024], f16,
                                          kind="ExternalInput")
    # bias image pre-broadcast for PSUM preload: [128, L, 8, GROUP_MAX*128]
    bias_dram = nc.dram_tensor("bias_img", [128, L, 8, GROUP_MAX * 128], f16,
                               kind="ExternalInput")
    out_t = nc.dram_tensor("out_t", [KC, 128, 128], f32, kind="ExternalOutput")

    with tile.TileContext(nc) as tc, ExitStack() as ctx:
        consts = ctx.enter_context(tc.tile_pool(name="consts", bufs=1))
        ps = ctx.enter_context(tc.tile_pool(name="ps", bufs=2, space="PSUM"))
        gp = ctx.enter_context(tc.tile_pool(name="gp", bufs=2))

        # --- input DMAs, latency-critical first ---
        bias_sb = consts.tile([128, L, 8, GROUP_MAX * 128], f16, tag="bias")
        nc.sync.dma_start(out=bias_sb[:], in_=bias_dram[:])
        slot0_h = consts.tile([128, L, KC, 128], f16, tag="slot0h")
        nc.sync.dma_start(out=slot0_h[:], in_=h0_t[:])
        slot0_c = consts.tile([128, L, KC, 128], f32, tag="slot0c")
        nc.sync.dma_start(out=slot0_c[:], in_=c0_t[:])
        xall = consts.tile([128, KC, nl0, 128], f16, tag="xall")
        nc.sync.dma_start(out=xall[:], in_=dags_t[:])
        wsb = {}
        for key in [("x", 0), ("h", 0), ("h", 1), ("x", 1)]:
            t = consts.tile([128, KC, 1024], f16, tag=f"w{key[0]}{key[1]}",
                            name=f"w{key[0]}{key[1]}")
            nc.gpsimd.dma_start(out=t[:], in_=w_dram[key][:])
            wsb[key] = t

        # --- PE warmup: dummy matmuls while input DMAs land ---
        warm = consts.tile([128, 128], f16, tag="warm")
        nc.vector.memset(warm[:], 0.0)
        wpt = ps.tile([128, 8, GROUP_MAX * 128], f32, tag="gates",
                      name="gates")
        for _ in range(20):
            nc.tensor.matmul(out=wpt[:, 0, :128], lhsT=warm[:], rhs=warm[:],
                             start=True, stop=True, skip_group_check=True)

        # per-group persistent state tiles; units are views [:, j]
        st_h = {}
        st_c = {}
        for g, (l, nodes) in enumerate(groups):
            u = len(nodes)
            gh = consts.tile([128, u, KC, 128], f16, tag=f"gh{g}",
                             name=f"gh{g}")
            gc = consts.tile([128, u, KC, 128], f32, tag=f"gc{g}",
                             name=f"gc{g}")
            for j, i in enumerate(nodes):
                st_h[(i, l)] = gh[:, j]
                st_c[(i, l)] = gc[:, j]
            groups[g] = (l, nodes, gh, gc)
        outh = consts.tile([128, KC, 128], f32, tag="outh")

        def h_ap(v, l):
            if v == 0:
                return slot0_h[:, l]
            return st_h[(v - 1, l)]

        def c_ap(v, l):
            if v == 0:
                return slot0_c[:, l]
            return st_c[(v - 1, l)]

        for (l, nodes, gh, gc) in groups:
            u = len(nodes)
            un = u * 128
            # one contiguous psum tile for all 8 gate chunks
            pt = ps.tile([128, 8, GROUP_MAX * 128], f32, tag="gates",
                         name="gates")
            # PSUM preload: bias (matmuls accumulate with start=False)
            nc.vector.tensor_copy(out=pt[:, :, :un],
                                  in_=bias_sb[:, l, :, :un])

            xq = gp.tile([128, KC, u, 128], f16, tag="xq", name="xq")
            ubh = gp.tile([128, KC, u, 128], f16, tag="ubh")
            ubc = gp.tile([128, KC, u, 128], f32, tag="ubc")
            sifo = gp.tile([128, 4, u, 128], f16, tag="sifo")
            gt = gp.tile([128, KC, u, 128], f16, tag="gt")
            so = gp.tile([128, KC, u, 128], f16, tag="so")
            vw = gp.tile([128, KC, u, 128], f16, tag="vw")
            cf = gp.tile([128, KC, u, 128], f32, tag="cf")
            th = gp.tile([128, u, KC, 128], f16, tag="th")

            # 1. layer-1 input: copy of h_l0 (layer 0 reads xall direct)
            if l == 1:
                for j, i in enumerate(nodes):
                    nc.vector.tensor_copy(out=xq[:, :, j, :],
                                          in_=st_h[(i, 0)])
            xcol = l0_col[nodes[0]] if l == 0 else 0

            # 2. predecessor state sums
            for j, i in enumerate(nodes):
                a, b_ = int(pred[i][0]), int(pred[i][1])
                if a == b_:
                    nc.vector.tensor_scalar_mul(ubh[:, :, j, :], h_ap(a, l),
                                                2.0)
                    nc.vector.tensor_scalar_mul(ubc[:, :, j, :], c_ap(a, l),
                                                2.0)
                else:
                    nc.vector.tensor_tensor(out=ubh[:, :, j, :],
                                            in0=h_ap(a, l), in1=h_ap(b_, l),
                                            op=Alu.add)
                    nc.vector.tensor_tensor(out=ubc[:, :, j, :],
                                            in0=c_ap(a, l), in1=c_ap(b_, l),
                                            op=Alu.add)

            # 3. gate matmuls accumulate onto the preload (start=False)
            def x_rhs(k):
                if l == 0:
                    return xall[:, k, xcol:xcol + u].rearrange(
                        "p u b -> p (u b)")
                return xq[:, k].rearrange("p u b -> p (u b)")
            for m in range(8):
                for oi in range(2):
                    for k in range(KC):
                        nc.tensor.matmul(
                            out=pt[:, m, :un],
                            lhsT=wsb[("x" if oi == 0 else "h", l)]
                            [:, k, m * 128:(m + 1) * 128],
                            rhs=x_rhs(k) if oi == 0 else
                            ubh[:, k].rearrange("p u b -> p (u b)"),
                            start=False, stop=(oi == 1 and k == KC - 1),
                            skip_group_check=True)

            # 4. merged activations: sigmoid(i,f), tanh(g), sigmoid(o)
            nc.scalar.activation(out=sifo[:].rearrange("p c u b -> p c (u b)"),
                                 in_=pt[:, 0:4, :un], func=AF.Sigmoid)
            nc.scalar.activation(out=gt[:].rearrange("p c u b -> p c (u b)"),
                                 in_=pt[:, 4:6, :un], func=AF.Tanh)
            nc.scalar.activation(out=so[:].rearrange("p c u b -> p c (u b)"),
                                 in_=pt[:, 6:8, :un], func=AF.Sigmoid)

            # 5. c_new = (ubc*0.5)*sigmoid(f) + sigmoid(i)*tanh(g)
            nc.vector.tensor_tensor(out=vw[:], in0=sifo[:, 0:2], in1=gt[:],
                                    op=Alu.mult)
            nc.vector.scalar_tensor_tensor(
                out=cf[:], in0=ubc[:], scalar=0.5, in1=sifo[:, 2:4],
                op0=Alu.mult, op1=Alu.mult)
            nc.vector.tensor_tensor(out=gc[:].rearrange("p u c b -> p c u b"),
                                    in0=cf[:], in1=vw[:], op=Alu.add)

            # 6. h = sigmoid(o) * tanh(c)
            nc.scalar.activation(out=th[:].rearrange("p u c b -> p (u c b)"),
                                 in_=gc[:].rearrange("p u c b -> p (u c b)"),
                                 func=AF.Tanh)
            nc.vector.tensor_tensor(out=gh[:].rearrange("p u c b -> p c u b"),
                                    in0=so[:],
                                    in1=th[:].rearrange("p u c b -> p c u b"),
                                    op=Alu.mult)
            if l == 1 and N - 1 in nodes:
                j = nodes.index(N - 1)
                nc.vector.tensor_tensor(out=outh[:], in0=so[:, :, j, :],
                                        in1=th[:, j], op=Alu.mult)

        # output: h of last node, top layer: [128, KC, 128] -> [KC, 128, 128]
        nc.sync.dma_start(out=out_t.ap().rearrange("k p b -> p k b"),
                          in_=outh[:])

    nc.compile()
    return nc, l0_nodes


def _get_program(pred):
    key = pred.tobytes()
    if key not in _CACHE:
        _CACHE[key] = _build_program(pred)
    return _CACHE[key]


def _prepare(dags, h0, c0, w_ih0, w_hh0, b_ih0, b_hh0,
             w_ih1, w_hh1, b_ih1, b_hh1, pred_idx):
    """Host-side prep: returns (nc, in_maps)."""
    dags = np.asarray(dags, dtype=np.float32)
    h0 = np.asarray(h0, dtype=np.float32)
    c0 = np.asarray(c0, dtype=np.float32)
    pred = np.asarray(pred_idx)

    nc, l0_nodes = _get_program(pred)

    wx0, wh0 = _prep_weights(np.asarray(w_ih0, np.float32),
                             np.asarray(w_hh0, np.float32))
    wx1, wh1 = _prep_weights(np.asarray(w_ih1, np.float32),
                             np.asarray(w_hh1, np.float32))
    bias = np.stack([
        np.asarray(b_ih0, np.float32) + np.asarray(b_hh0, np.float32),
        np.asarray(b_ih1, np.float32) + np.asarray(b_hh1, np.float32),
    ])  # [L, 1024]
    # bias image [128, L, 8, GROUP_MAX*128]: row (chunk*128 + p) broadcast
    bimg = np.ascontiguousarray(
        np.broadcast_to(
            bias.reshape(L, 8, 128).transpose(2, 0, 1)[:, :, :, None],
            (128, L, 8, GROUP_MAX * 128)).astype(np.float16))

    in_maps = []
    for c in range(NCORES):
        bs = slice(c * BL, (c + 1) * BL)
        # dags [B, N, IN] -> cone l0 nodes (group order) ->
        # [128(p), KC, n, 128(b)] fp16
        sel = dags[bs][:, l0_nodes]                      # [BL, n, IN]
        dt_ = sel.transpose(2, 1, 0).reshape(KC, 128, len(l0_nodes), BL)
        dt_ = np.ascontiguousarray(
            dt_.transpose(1, 0, 2, 3).astype(np.float16))
        # h0/c0 [L, B, H] -> [128(p), L, kc, b]
        hh = h0[:, bs, :].transpose(2, 0, 1).reshape(KC, 128, L, BL)
        cc = c0[:, bs, :].transpose(2, 0, 1).reshape(KC, 128, L, BL)
        h0t = np.ascontiguousarray(
            hh.transpose(1, 2, 0, 3).astype(np.float16))  # [128, L, kc, b]
        c0t = np.ascontiguousarray(
            cc.transpose(1, 2, 0, 3).astype(np.float32))
        in_maps.append({
            "dags_t": dt_, "h0_t": h0t, "c0_t": c0t,
            "wx0": wx0, "wh0": wh0, "wx1": wx1, "wh1": wh1,
            "bias_img": bimg,
        })
    return nc, in_maps


def _assemble(res):
    out = np.empty((B, H), np.float32)
    for c in range(NCORES):
        ot = res.results[c]["out_t"]  # [KC, 128, 128] = [kc, p, b]
        out[c * BL:(c + 1) * BL] = ot.reshape(H, BL).T
    return out


def kernel(**inputs):
    from concourse.bass_utils import run_bass_kernel_spmd

    nc, in_maps = _prepare(**inputs)
    res = run_bass_kernel_spmd(nc, in_maps, list(range(NCORES)))
    return _assemble(res)


# revision 27
# speedup vs baseline: 1.1796x; 1.0465x over previous
"""DAG-LSTM Trainium2 kernel.

Problem: 2-layer LSTM scanned over a 48-node DAG, batch 1024, hidden 256.
Sharding: pure data parallelism -- batch split 8 x 128 across NeuronCores,
weights replicated, no cross-device traffic.

Key optimizations over the naive scan:
1. Dead-code elimination: the reference returns only the top-layer hidden
   state of the LAST DAG node, so only the ancestor cone of (node 47,
   layer 1) is computed -- 20 of 96 (node, layer) units for this graph.
2. The layer-0 input-side gate contributions (W_ih0 @ x_i) have no DAG
   dependency; they are precomputed in wide batched matmuls during the
   input-DMA head (also warming up the PE) and stashed in SBUF fp32.
   Per-group PSUM is then preloaded with bias + x-gates in one DVE op and
   the in-loop matmuls are h-side only (start=False accumulation).
3. Merged activations: one contiguous PSUM tile [128, 8 chunks, u*128] per
   group => sigmoid(i,f) / tanh(g) / sigmoid(o) are three instructions.
4. c-path kept fp32 (f16 c on HW costs ~1.3e-2 rel err; fp32 gives 9e-4).

Layout: "transposed" (feature-on-partition): states h [2x128 part-chunks,
B=128 free] f16, c fp32.
"""

import sys
import numpy as np

sys.path.insert(0, "/opt/trn_rl_repo")

B, N, IN, H, L, P = 1024, 48, 256, 256, 2, 2
NCORES = 8
BL = B // NCORES          # 128 batch per core
KC = 2                    # K chunks (256 = 2*128)
GROUP_MAX = 2             # units per (stage, layer) group
PCW = 2                   # precompute wave width (nodes per wave)

_CACHE = {}


def _unit_deps(pred, i, l):
    d = [(int(v) - 1, l) for v in pred[i] if v > 0]
    if l == 1:
        d.append((i, 0))
    return d


def _build_schedule(pred):
    """Ancestor cone of (N-1, 1) scheduled into ASAP stages; same-stage
    same-layer units grouped up to GROUP_MAX.  Returns list of
    (layer, [nodes]) in dependency order."""
    cone = set()
    stack = [(N - 1, 1)]
    while stack:
        u = stack.pop()
        if u in cone:
            continue
        cone.add(u)
        stack.extend(_unit_deps(pred, *u))
    stage = {}
    for u in sorted(cone):
        ds = [d for d in _unit_deps(pred, *u) if d in cone]
        stage[u] = max([stage[d] for d in ds], default=-1) + 1
    nstages = max(stage.values()) + 1
    groups = []
    for s in range(nstages):
        for l in (0, 1):
            nodes = sorted(i for (i, ll), st in stage.items()
                           if st == s and ll == l)
            for k in range(0, len(nodes), GROUP_MAX):
                groups.append((l, nodes[k:k + GROUP_MAX]))
    return groups


def _prep_weights(w_ih, w_hh):
    """Host-side weight prep -> [128, KC, 1024] fp16 stationary tiles.
    The 0.5 predecessor-mean is folded into W_hh."""

    def to_t(w):
        kdim = w.shape[1]
        wt = np.ascontiguousarray(w.T)            # [K, 1024]
        wt = wt.reshape(kdim // 128, 128, 1024)   # [kc, kin, 1024]
        return np.ascontiguousarray(
            wt.transpose(1, 0, 2).astype(np.float16))  # [128, kc, 1024]

    return to_t(w_ih), to_t(w_hh * 0.5)


def _build_program(pred):
    from contextlib import ExitStack
    from concourse import bacc, mybir, tile

    f32 = mybir.dt.float32
    f16 = mybir.dt.float16
    AF = mybir.ActivationFunctionType
    Alu = mybir.AluOpType

    groups = _build_schedule(pred)
    # layer-0 nodes in group order -> column order of the precomputed
    # x-gates stash (so per-group slices are contiguous)
    l0_nodes = [i for (l, nodes) in groups if l == 0 for i in nodes]
    l0_col = {i: k for k, i in enumerate(l0_nodes)}
    nl0 = len(l0_nodes)

    nc = bacc.Bacc("TRN2", target_bir_lowering=False, debug=False,
                   num_devices=NCORES)

    # dags gathered+transposed on host: [128, KC, nl0, 128] (group order)
    dags_t = nc.dram_tensor("dags_t", [128, KC, nl0, 128], f16,
                            kind="ExternalInput")
    h0_t = nc.dram_tensor("h0_t", [128, L, KC, 128], f16,
                          kind="ExternalInput")
    c0_t = nc.dram_tensor("c0_t", [128, L, KC, 128], f32,
                          kind="ExternalInput")
    w_dram = {}
    for l in range(L):
        w_dram[("x", l)] = nc.dram_tensor(f"wx{l}", [128, KC, 1024], f16,
                                          kind="ExternalInput")
        w_dram[("h", l)] = nc.dram_tensor(f"wh{l}", [128, KC, 1024], f16,
                                          kind="ExternalInput")
    # bias images pre-broadcast for PSUM preload, one per layer
    bias_dram = [nc.dram_tensor(f"bias_img{l}", [128, 8, 128], f16,
                                kind="ExternalInput") for l in range(L)]
    out_t = nc.dram_tensor("out_t", [KC, 128, 128], f32, kind="ExternalOutput")

    with tile.TileContext(nc) as tc, ExitStack() as ctx:
        consts = ctx.enter_context(tc.tile_pool(name="consts", bufs=1))
        ps = ctx.enter_context(tc.tile_pool(name="ps", bufs=2, space="PSUM"))
        gp = ctx.enter_context(tc.tile_pool(name="gp", bufs=2))

        # --- input DMAs, latency-critical first ---
        bias_sb = [consts.tile([128, 8, 128], f16, tag=f"bias{l}",
                               name=f"bias{l}") for l in range(L)]
        nc.sync.dma_start(out=bias_sb[0][:], in_=bias_dram[0][:])
        slot0_h = consts.tile([128, L, KC, 128], f16, tag="slot0h")
        nc.sync.dma_start(out=slot0_h[:], in_=h0_t[:])
        slot0_c = consts.tile([128, L, KC, 128], f32, tag="slot0c")
        nc.sync.dma_start(out=slot0_c[:], in_=c0_t[:])
        nc.sync.dma_start(out=bias_sb[1][:], in_=bias_dram[1][:])
        xall = consts.tile([128, KC, nl0, 128], f16, tag="xall")
        nc.sync.dma_start(out=xall[:], in_=dags_t[:])
        wsb = {}
        for key in [("x", 0), ("h", 0), ("h", 1), ("x", 1)]:
            t = consts.tile([128, KC, 1024], f16, tag=f"w{key[0]}{key[1]}",
                            name=f"w{key[0]}{key[1]}")
            nc.gpsimd.dma_start(out=t[:], in_=w_dram[key][:])
            wsb[key] = t

        # --- PE warmup: dummy matmuls while input DMAs land ---
        warm = consts.tile([128, 256], f16, tag="warm")
        nc.vector.memset(warm[:], 0.0)
        wpt = ps.tile([128, 8, GROUP_MAX * 128], f32, tag="gates",
                      name="gates")
        for _ in range(40):
            nc.tensor.matmul(out=wpt[:, 0, :256], lhsT=warm[:, :128],
                             rhs=warm[:], start=True, stop=True,
                             skip_group_check=True)

        # per-group persistent state tiles; units are views [:, j]
        st_h = {}
        st_c = {}
        for g, (l, nodes) in enumerate(groups):
            u = len(nodes)
            gh = consts.tile([128, u, KC, 128], f16, tag=f"gh{g}",
                             name=f"gh{g}")
            gc = consts.tile([128, u, KC, 128], f32, tag=f"gc{g}",
                             name=f"gc{g}")
            for j, i in enumerate(nodes):
                st_h[(i, l)] = gh[:, j]
                st_c[(i, l)] = gc[:, j]
            groups[g] = (l, nodes, gh, gc)
        outh = consts.tile([128, KC, 128], f32, tag="outh")

        def h_ap(v, l):
            if v == 0:
                return slot0_h[:, l]
            return st_h[(v - 1, l)]

        def c_ap(v, l):
            if v == 0:
                return slot0_c[:, l]
            return st_c[(v - 1, l)]

        for (l, nodes, gh, gc) in groups:
            u = len(nodes)
            un = u * 128
            # one contiguous psum tile for all 8 gate chunks
            pt = ps.tile([128, 8, GROUP_MAX * 128], f32, tag="gates",
                         name="gates")
            # PSUM preload on the scalar engine (idle slot between groups;
            # keeps the DVE critical chain clear)
            for j in range(u):
                nc.scalar.copy(out=pt[:, :, j * 128:(j + 1) * 128],
                               in_=bias_sb[l][:])

            xq = gp.tile([128, KC, u, 128], f16, tag="xq", name="xq")
            ubh = gp.tile([128, KC, u, 128], f16, tag="ubh")
            ubc = gp.tile([128, KC, u, 128], f32, tag="ubc")
            sifo = gp.tile([128, 4, u, 128], f16, tag="sifo")
            gt = gp.tile([128, KC, u, 128], f16, tag="gt")
            so = gp.tile([128, KC, u, 128], f16, tag="so")
            vw = gp.tile([128, KC, u, 128], f16, tag="vw")
            cf = gp.tile([128, KC, u, 128], f32, tag="cf")
            th = gp.tile([128, u, KC, 128], f16, tag="th")

            # 1. layer-1 input: copy of h_l0 (layer 0 reads xall direct)
            if l == 1:
                for j, i in enumerate(nodes):
                    nc.vector.tensor_copy(out=xq[:, :, j, :],
                                          in_=st_h[(i, 0)])
            xcol = l0_col[nodes[0]] if l == 0 else 0

            # 2. predecessor state sums
            for j, i in enumerate(nodes):
                a, b_ = int(pred[i][0]), int(pred[i][1])
                if a == b_:
                    nc.vector.tensor_scalar_mul(ubh[:, :, j, :], h_ap(a, l),
                                                2.0)
                    nc.vector.tensor_scalar_mul(ubc[:, :, j, :], c_ap(a, l),
                                                2.0)
                else:
                    nc.vector.tensor_tensor(out=ubh[:, :, j, :],
                                            in0=h_ap(a, l), in1=h_ap(b_, l),
                                            op=Alu.add)
                    nc.vector.tensor_tensor(out=ubc[:, :, j, :],
                                            in0=c_ap(a, l), in1=c_ap(b_, l),
                                            op=Alu.add)

            # 3. gate matmuls accumulate onto the preload (start=False)
            def x_rhs(k):
                if l == 0:
                    return xall[:, k, xcol:xcol + u].rearrange(
                        "p u b -> p (u b)")
                return xq[:, k].rearrange("p u b -> p (u b)")
            for m in range(8):
                for oi in range(2):
                    for k in range(KC):
                        nc.tensor.matmul(
                            out=pt[:, m, :un],
                            lhsT=wsb[("x" if oi == 0 else "h", l)]
                            [:, k, m * 128:(m + 1) * 128],
                            rhs=x_rhs(k) if oi == 0 else
                            ubh[:, k].rearrange("p u b -> p (u b)"),
                            start=False, stop=(oi == 1 and k == KC - 1),
                            skip_group_check=True)

            # 4. merged activations: sigmoid(i,f), tanh(g), sigmoid(o)
            nc.scalar.activation(out=sifo[:].rearrange("p c u b -> p c (u b)"),
                                 in_=pt[:, 0:4, :un], func=AF.Sigmoid)
            nc.scalar.activation(out=gt[:].rearrange("p c u b -> p c (u b)"),
                                 in_=pt[:, 4:6, :un], func=AF.Tanh)
            nc.scalar.activation(out=so[:].rearrange("p c u b -> p c (u b)"),
                                 in_=pt[:, 6:8, :un], func=AF.Sigmoid)

            # 5. c_new = (ubc*0.5)*sigmoid(f) + sigmoid(i)*tanh(g)
            nc.vector.tensor_tensor(out=vw[:], in0=sifo[:, 0:2], in1=gt[:],
                                    op=Alu.mult)
            nc.vector.scalar_tensor_tensor(
                out=cf[:], in0=ubc[:], scalar=0.5, in1=sifo[:, 2:4],
                op0=Alu.mult, op1=Alu.mult)
            nc.vector.tensor_tensor(out=gc[:].rearrange("p u c b -> p c u b"),
                                    in0=cf[:], in1=vw[:], op=Alu.add)

            # 6. h = sigmoid(o) * tanh(c)
            nc.scalar.activation(out=th[:].rearrange("p u c b -> p (u c b)"),
                                 in_=gc[:].rearrange("p u c b -> p (u c b)"),
                                 func=AF.Tanh)
            nc.vector.tensor_tensor(out=gh[:].rearrange("p u c b -> p c u b"),
                                    in0=so[:],
                                    in1=th[:].rearrange("p u c b -> p c u b"),
                                    op=Alu.mult)
            if l == 1 and N - 1 in nodes:
                j = nodes.index(N - 1)
                nc.vector.tensor_tensor(out=outh[:], in0=so[:, :, j, :],
                                        in1=th[:, j], op=Alu.mult)

        # output: h of last node, top layer: [128, KC, 128] -> [KC, 128, 128]
        nc.sync.dma_start(out=out_t.ap().rearrange("k p b -> p k b"),
                          in_=outh[:])

    nc.compile()
    return nc, l0_nodes


def _get_program(pred):
    key = pred.tobytes()
    if key not in _CACHE:
        _CACHE[key] = _build_program(pred)
    return _CACHE[key]


def _prepare(dags, h0, c0, w_ih0, w_hh0, b_ih0, b_hh0,
             w_ih1, w_hh1, b_ih1, b_hh1, pred_idx):
    """Host-side prep: returns (nc, in_maps)."""
    dags = np.asarray(dags, dtype=np.float32)
    h0 = np.asarray(h0, dtype=np.float32)
    c0 = np.asarray(c0, dtype=np.float32)
    pred = np.asarray(pred_idx)

    nc, l0_nodes = _get_program(pred)

    wx0, wh0 = _prep_weights(np.asarray(w_ih0, np.float32),
                             np.asarray(w_hh0, np.float32))
    wx1, wh1 = _prep_weights(np.asarray(w_ih1, np.float32),
                             np.asarray(w_hh1, np.float32))
    bias = np.stack([
        np.asarray(b_ih0, np.float32) + np.asarray(b_hh0, np.float32),
        np.asarray(b_ih1, np.float32) + np.asarray(b_hh1, np.float32),
    ])  # [L, 1024]
    # bias images [128, 8, 128] per layer: row (chunk*128 + p) broadcast
    bimg = np.ascontiguousarray(
        np.broadcast_to(
            bias.reshape(L, 8, 128).transpose(2, 0, 1)[:, :, :, None],
            (128, L, 8, 128)).astype(np.float16))

    in_maps = []
    for c in range(NCORES):
        bs = slice(c * BL, (c + 1) * BL)
        # dags [B, N, IN] -> cone l0 nodes (group order) ->
        # [128(p), KC, n, 128(b)] fp16
        sel = dags[bs][:, l0_nodes]                      # [BL, n, IN]
        dt_ = sel.transpose(2, 1, 0).reshape(KC, 128, len(l0_nodes), BL)
        dt_ = np.ascontiguousarray(
            dt_.transpose(1, 0, 2, 3).astype(np.float16))
        # h0/c0 [L, B, H] -> [128(p), L, kc, b]
        hh = h0[:, bs, :].transpose(2, 0, 1).reshape(KC, 128, L, BL)
        cc = c0[:, bs, :].transpose(2, 0, 1).reshape(KC, 128, L, BL)
        h0t = np.ascontiguousarray(
            hh.transpose(1, 2, 0, 3).astype(np.float16))  # [128, L, kc, b]
        c0t = np.ascontiguousarray(
            cc.transpose(1, 2, 0, 3).astype(np.float32))
        in_maps.append({
            "dags_t": dt_, "h0_t": h0t, "c0_t": c0t,
            "wx0": wx0, "wh0": wh0, "wx1": wx1, "wh1": wh1,
            "bias_img0": bimg[:, 0], "bias_img1": bimg[:, 1],
        })
    return nc, in_maps


def _assemble(res):
    out = np.empty((B, H), np.float32)
    for c in range(NCORES):
        ot = res.results[c]["out_t"]  # [KC, 128, 128] = [kc, p, b]
        out[c * BL:(c + 1) * BL] = ot.reshape(H, BL).T
    return out


def kernel(**inputs):
    from concourse.bass_utils import run_bass_kernel_spmd

    nc, in_maps = _prepare(**inputs)
    res = run_bass_kernel_spmd(nc, in_maps, list(range(NCORES)))
    return _assemble(res)
